# revision 1
# baseline (speedup 1.0000x reference)
"""Trainium2 Bass kernel for nn_CrossAttention_31791347925417.

Math (per batch b, per stream tok in {x, blood} with weight W in {W1, W2}):
    kv = tok @ W.T ; k, v heads [H, N, D]
    ctx = softmax_d( SCALE * k_h^T v_h )          # [H, D, D], softmax over first D
    out_x = x_h @ ctx2_h ; out_b = blood_h @ ctx1_h

Refactor used here (Gram trick):
    k_h^T v_h = W_k_h (tok^T tok) W_v_h^T  with G = tok^T tok  [C, C]
so the N=4096 contraction happens once (G) instead of twice (k and v), and
everything downstream is tiny [C,C]-scale work.

We compute ctxT_h = (SCALE*W_k applied) via  Q = G @ WkT, then per head-pair
a full [128,128] product  WvT_pair^T @ Q_pair  whose DIAGONAL 64x64 blocks are
ctxT_h [e, d] for the two heads (off-diagonal blocks are unused).  Softmax runs
along the free axis (d).  The normalized probs are written into the diagonal
blocks of a zeroed [128,128] tile F; BD = F^T (PE transpose) is the
block-diagonal ctx pair used by the output matmuls:
    out[n, (h,e)] = sum_{(h,d)} xT[(h,d), n] * BD[(h,d), (h,e)]

Sharding: data-parallel over batch B=8 across the 8 cores; weights replicated.
Host pre-transposes W -> W.T [C, 2C] and folds SCALE into the k-half (exact,
SCALE = 0.125).
"""

import sys

if "/opt/trn_rl_repo" not in sys.path:
    sys.path.insert(0, "/opt/trn_rl_repo")

import numpy as np

from concourse import bacc, masks, mybir, tile
from concourse.bass_utils import run_bass_kernel_spmd

B, N, C, H = 8, 4096, 512, 8
D = C // H
SCALE = D ** -0.5
P = 128
NBIG = N // 512          # 8 big row tiles (512 rows each)
NT = N // P              # 32 n-tiles
CB = C // P              # 4 column blocks == head pairs
F32 = mybir.dt.float32
F32R = mybir.dt.float32r
BF16 = mybir.dt.bfloat16
AX = mybir.AxisListType
ACT_EXP = mybir.ActivationFunctionType.Exp

# precision knobs
G_F32R = True      # G = tok^T tok in float32r (4x faster than float32)
Q_F32R = True      # Q = G @ WkT in float32r
OUT_BF16 = True    # final out matmuls in bf16 (vs float32)
TRANS_BF16_ID = False  # walrus rejects mixed f32r/bf16 matmul operands
TRANS_F32R = False  # walrus codegen rejects f32r transpose-mode


def _r(ap):
    return ap.bitcast(F32R)


def build_nc():
    nc = bacc.Bacc("TRN2", target_bir_lowering=False, debug=False)

    TOKDT = F32R if G_F32R else F32
    WDT = F32R if Q_F32R else F32
    xb = nc.dram_tensor("xb", [N, C], TOKDT, kind="ExternalInput").ap()
    bb = nc.dram_tensor("bb", [N, C], TOKDT, kind="ExternalInput").ap()
    w1t = nc.dram_tensor("w1t", [C, 2 * C], WDT, kind="ExternalInput").ap()
    w2t = nc.dram_tensor("w2t", [C, 2 * C], WDT, kind="ExternalInput").ap()
    # blocked transposed output layout: [kb, part(c within pair), pair, n-col]
    ox = nc.dram_tensor("oxT", [NBIG, P, CB, 512], F32, kind="ExternalOutput").ap()
    ob = nc.dram_tensor("obT", [NBIG, P, CB, 512], F32, kind="ExternalOutput").ap()

    with tile.TileContext(nc) as tc:
        _emit(nc, tc, xb, bb, w1t, w2t, ox, ob)

    nc.compile()
    return nc


def _emit(nc, tc, xb, bb, w1t, w2t, ox, ob):
    TOKDT = F32R if G_F32R else F32
    WDT = F32R if Q_F32R else F32
    from contextlib import ExitStack

    ctx = ExitStack()
    with ctx:
        const = ctx.enter_context(tc.tile_pool(name="const", bufs=1))
        wpool = ctx.enter_context(tc.tile_pool(name="wpool", bufs=1))
        tokp = ctx.enter_context(tc.tile_pool(name="tokp", bufs=8))
        xtp = ctx.enter_context(tc.tile_pool(name="xtp", bufs=1))
        gqp = ctx.enter_context(tc.tile_pool(name="gqp", bufs=8))
        smallp = ctx.enter_context(tc.tile_pool(name="smallp", bufs=2))
        fpool = ctx.enter_context(tc.tile_pool(name="fpool", bufs=2))
        bdpool = ctx.enter_context(tc.tile_pool(name="bdpool", bufs=8))
        ostp = ctx.enter_context(tc.tile_pool(name="ostp", bufs=3))
        psG = ctx.enter_context(tc.tile_pool(name="psG", bufs=4, space="PSUM"))
        psT = ctx.enter_context(tc.tile_pool(name="psT", bufs=2, space="PSUM"))
        psO = ctx.enter_context(tc.tile_pool(name="psO", bufs=2, space="PSUM"))

        ident = const.tile([P, P], F32, tag="idf")
        masks.make_identity(nc, ident[:])
        ident_bf = const.tile([P, P], BF16, tag="idb")
        masks.make_identity(nc, ident_bf[:])
        if TRANS_F32R:
            ident_r = const.tile([P, P], F32R, tag="idr")
            masks.make_identity(nc, ident_r[:])
        else:
            ident_r = None

        # weights: chunk j (c-rows 128j..128j+128) lives at cols [j*2C, (j+1)*2C)
        w_x = wpool.tile([P, CB * 2 * C], WDT, tag="wx")
        w_b = wpool.tile([P, CB * 2 * C], WDT, tag="wb")

        def load_weights():
            nc.sync.dma_start(
                w_x[:].rearrange("p (j c) -> p j c", j=CB),
                w1t[:, :].rearrange("(j p) c -> p j c", p=P),
            )
            nc.sync.dma_start(
                w_b[:].rearrange("p (j c) -> p j c", j=CB),
                w2t[:, :].rearrange("(j p) c -> p j c", p=P),
            )

        def wchunk(w, j):
            return w[:, j * 2 * C:(j + 1) * 2 * C]

        # transposed tokens, bf16: pair block m at cols [m*N, (m+1)*N)
        xT_x = xtp.tile([P, CB * N], BF16, tag="xtx")
        xT_b = xtp.tile([P, CB * N], BF16, tag="xtb")

        def emit_loads(tok_dram, split_first=False):
            toks = []
            for kb in range(NBIG):
                tokb = tokp.tile([P, 4 * C], TOKDT, tag="tok", name=f"tok{kb}")
                if kb == 0 and split_first:
                    for sub in range(4):
                        nc.sync.dma_start(
                            tokb[:, sub * C:(sub + 1) * C],
                            tok_dram[kb * 512 + sub * P:kb * 512 + (sub + 1) * P, :],
                        )
                else:
                    nc.sync.dma_start(
                        tokb[:].rearrange("p (s c) -> p s c", s=4),
                        tok_dram[kb * 512:(kb + 1) * 512, :].rearrange(
                            "(s p) c -> p s c", p=P
                        ),
                    )
                toks.append(tokb)
            return toks

        # G is symmetric: row-block m only needs columns >= G_OFF[m]
        # (row 3 starts at 256 to keep the f32r moving dim >= 256).
        G_OFF = [0, P, 2 * P, 2 * P]

        def emit_G_tile(gps, sb, k):
            for m in range(CB):
                o = G_OFF[m]
                nc.tensor.matmul(
                    gps[m][:, o:C], sb[:, m * P:(m + 1) * P], sb[:, o:C],
                    start=(k == 0), stop=(k == NT - 1),
                )

        def emit_T_tile(xT, sb, k, alt):
            tpool, ttag = (psT, "t") if (not alt or k % 2 == 0) else (psO, "o")
            tps = tpool.tile([P, C], F32, tag=ttag, name="tps")
            for m in range(CB):
                nc.tensor.transpose(
                    tps[:, m * P:(m + 1) * P],
                    sb[:, m * P:(m + 1) * P].bitcast(F32), ident[:],
                )
            nc.vector.tensor_copy(
                xT[:].rearrange("p (m n) -> p m n", m=CB)[:, :, k * P:(k + 1) * P],
                tps[:].rearrange("p (m n) -> p m n", m=CB),
            )

        def out_chunk(xT, BDs, kb, ost, pool, ptag):
            """outT for 512 n-cols (tile-group kb): per pair p one matmul
            [c-block p, 512 n]; drain into ost quarter p (split DVE/ACT)."""
            for p in range(CB):
                ops = pool.tile([P, 512], F32, tag=ptag, name=f"ops{p}")
                nc.tensor.matmul(
                    ops[:], BDs[p][:], xT[:, p * N + kb * 512:p * N + (kb + 1) * 512],
                    start=True, stop=True,
                )
                if p % 2:
                    nc.scalar.copy(ost[:, p * 512:(p + 1) * 512], ops[:])
                else:
                    nc.vector.tensor_copy(ost[:, p * 512:(p + 1) * 512], ops[:])

        def emit_chain(gps, w):
            """G psum -> Q -> per-pair ctxT blocks -> softmax -> BD tiles.
            Q/ctx psums live in the "o" pool so the next stream's G can claim
            the "g" banks immediately."""
            g_sb = []
            for m in range(CB):
                o = G_OFF[m]
                g = gqp.tile([P, C], WDT, tag="gq", name=f"g{m}")
                nc.scalar.copy(g[:, o:C], gps[m][:, o:C])
                g_sb.append(g)
            # mirror missing lower blocks (i,j), j < G_OFF[i]//P, from (j,i)^T
            for i in range(CB):
                for j in range(G_OFF[i] // P):
                    mps = psT.tile([P, P], F32, tag="t", name="mps")
                    nc.tensor.transpose(
                        mps[:], g_sb[j][:, i * P:(i + 1) * P].bitcast(F32),
                        ident[:],
                    )
                    nc.scalar.copy(g_sb[i][:, j * P:(j + 1) * P], mps[:])

            q_sb = [None] * CB
            for i in reversed(range(CB)):
                qp = psO.tile([P, C], F32, tag="o", name=f"qp{i}")
                for j in range(CB):
                    nc.tensor.matmul(
                        qp[:], g_sb[j][:, i * P:(i + 1) * P],
                        wchunk(w, j)[:, 0:C], start=(j == 0), stop=(j == 3),
                    )
                q = gqp.tile([P, C], WDT, tag="gq", name=f"q{i}")
                nc.scalar.copy(q[:], qp[:])
                q_sb[i] = q

            BDs = []
            for p in range(CB):
                cps = psO.tile([P, P], F32, tag="o", name=f"cps{p}")
                for j in range(CB):
                    nc.tensor.matmul(
                        cps[:],
                        wchunk(w, j)[:, C + p * P:C + (p + 1) * P],
                        q_sb[j][:, p * P:(p + 1) * P],
                        start=(j == 0), stop=(j == 3),
                    )
                nm = smallp.tile([P, 1], F32, tag="nm", name="nm")
                sm = smallp.tile([P, 1], F32, tag="sm", name="sm")
                rv = smallp.tile([P, 1], F32, tag="rv", name="rv")
                pp = smallp.tile([P, D], F32, tag="pp", name="pp")
                fp = fpool.tile([P, P], BF16, tag="F", name="fp")
                nc.gpsimd.memset(fp[:], 0.0)
                for dd in range(2):
                    s0 = slice(dd * D, (dd + 1) * D)
                    blk = cps[s0, s0]
                    nc.vector.reduce_max(nm[s0, :], blk, axis=AX.X, negate=True)
                    nc.scalar.activation(
                        pp[s0, :], blk, ACT_EXP, bias=nm[s0, :], scale=1.0,
                        accum_out=sm[s0, :],
                    )
                nc.vector.reciprocal(rv[:], sm[:])
                for dd in range(2):
                    s0 = slice(dd * D, (dd + 1) * D)
                    nc.vector.tensor_scalar_mul(fp[s0, s0], pp[s0, :], rv[s0, :])
                bps = psT.tile([P, P], BF16, tag="t", name="bps")
                nc.tensor.transpose(bps[:, 0:P], fp[:], ident_bf[:])
                bd = bdpool.tile([P, P], BF16, tag="bd", name=f"bd{p}")
                nc.vector.tensor_copy(bd[:], bps[:, 0:P])
                BDs.append(bd)
            return BDs

        # ---- schedule ----
        toks_x = emit_loads(xb, split_first=True)
        load_weights()
        toks_b = emit_loads(bb)

        # phase A: interleaved G_x + T_x per tile (DMA-bound window)
        gps_x = [psG.tile([P, C], F32, tag="g", name=f"gpsx{m}") for m in range(CB)]
        for kb in range(NBIG):
            for sub in range(4):
                k = kb * 4 + sub
                sb = toks_x[kb][:, sub * C:(sub + 1) * C]
                emit_G_tile(gps_x, sb, k)
                emit_T_tile(xT_x, sb, k, alt=True)
        bd1 = emit_chain(gps_x, w_x)

        # B1: dense G_b (claims the "g" banks as soon as chain A drains them)
        gps_b = [psG.tile([P, C], F32, tag="g", name=f"gpsb{m}") for m in range(CB)]
        for kb in range(NBIG):
            for sub in range(4):
                k = kb * 4 + sub
                emit_G_tile(gps_b, toks_b[kb][:, sub * C:(sub + 1) * C], k)
        bd2 = emit_chain(gps_b, w_b)

        # B2: production loop — transpose blood, then both outputs per kb;
        # writes stream at full DMA rate from here on
        for kb in range(NBIG):
            for sub in range(4):
                k = kb * 4 + sub
                emit_T_tile(xT_b, toks_b[kb][:, sub * C:(sub + 1) * C], k, alt=False)
            ost_b = ostp.tile([P, 4 * 512], F32, tag="ost", name="ost_b")
            out_chunk(xT_b, bd1, kb, ost_b, psO, "o")
            nc.scalar.dma_start(ob[kb], ost_b[:])
            ost_x = ostp.tile([P, 4 * 512], F32, tag="ost", name="ost_x")
            out_chunk(xT_x, bd2, kb, ost_x, psG, "g")
            nc.scalar.dma_start(ox[kb], ost_x[:])


_NC_CACHE = None


def _get_nc():
    global _NC_CACHE
    if _NC_CACHE is None:
        _NC_CACHE = build_nc()
    return _NC_CACHE


def _prep_inputs(x, blood, W1, W2):
    x = np.ascontiguousarray(np.asarray(x, dtype=np.float32))
    blood = np.ascontiguousarray(np.asarray(blood, dtype=np.float32))
    w1t = np.ascontiguousarray(np.asarray(W1, dtype=np.float32).T)
    w2t = np.ascontiguousarray(np.asarray(W2, dtype=np.float32).T)
    w1t[:, :C] *= SCALE  # fold softmax scale into the k-projection (exact: 2^-3)
    w2t[:, :C] *= SCALE
    return [
        {"xb": x[b], "bb": blood[b], "w1t": w1t, "w2t": w2t} for b in range(B)
    ]


def _unshuffle(arr):
    """[NBIG, P, CB, 512] blocked-transposed -> [N, C] natural."""
    # arr[kb, part, p, col] = out[kb*512 + col, p*128 + part]
    return np.ascontiguousarray(
        arr.transpose(0, 3, 2, 1).reshape(N, C))


def kernel(x, blood, W1, W2, trace=False):
    nc = _get_nc()
    in_maps = _prep_inputs(x, blood, W1, W2)
    res = run_bass_kernel_spmd(nc, in_maps, core_ids=list(range(B)), trace=trace)
    out_x = np.stack([_unshuffle(res.results[b]["oxT"]) for b in range(B)])
    out_b = np.stack([_unshuffle(res.results[b]["obT"]) for b in range(B)])
    if trace:
        kernel.last_results = res
    return (out_x, out_b)



# revision 6
# speedup vs baseline: 1.3089x; 1.3089x over previous
"""Trainium2 Bass kernel for nn_CrossAttention_31791347925417.

Math (per batch b, stream tok in {x, blood} with weight W in {W1, W2}):
    kv = tok @ W.T ; k, v heads [H, N, D]
    ctx = softmax_d( SCALE * k_h^T v_h )          # [H, D, D], softmax over first D
    out_x = x_h @ ctx2_h ; out_b = blood_h @ ctx1_h

Gram trick: k_h^T v_h = W_k_h (tok^T tok) W_v_h^T with G = tok^T tok [C, C],
so the N=4096 contraction happens once per stream and everything downstream is
tiny [C,C]-scale work.

v2 layout: tokens are shipped and processed in BF16 (validated: rel_l2 ~ 4e-3
vs the 2e-2 budget), weights stay f32 (bf16 W alone costs ~1e-2). All PE
matmuls run at 1 cycle/row; G uses a perfect upper-triangle block schedule
(bf16 has no >=256 moving-dim constraint), the ctx matmuls use a 256-wide
moving window to stay on the f32r fast path, and outputs are written bf16 and
upcast on the host. Phase B pipelines out_b one kb behind the G_b/T_b tiles so
the PE never waits on the DVE psum drains.

Sharding: data-parallel over batch B=8 across the 8 cores; weights replicated.
Host pre-transposes W -> W.T [C, 2C] and folds SCALE into the k-half (exact,
SCALE = 0.125).
"""

import sys

if "/opt/trn_rl_repo" not in sys.path:
    sys.path.insert(0, "/opt/trn_rl_repo")

import ml_dtypes
import numpy as np

from concourse import bacc, masks, mybir, tile
from concourse.bass_utils import run_bass_kernel_spmd

B, N, C, H = 8, 4096, 512, 8
D = C // H
SCALE = D ** -0.5
P = 128
NBIG = N // 512          # 8 big row tiles (512 rows each)
NT = N // P              # 32 n-tiles
CB = C // P              # 4 column blocks == head pairs
F32 = mybir.dt.float32
F32R = mybir.dt.float32r
BF16 = mybir.dt.bfloat16
AX = mybir.AxisListType
ACT_EXP = mybir.ActivationFunctionType.Exp

# upper-triangle block schedule: row-block m computes cols [G_OFF[m], C)
G_OFF = [0, P, 2 * P, 3 * P]


def build_nc():
    nc = bacc.Bacc("TRN2", target_bir_lowering=False, debug=False)

    xb = nc.dram_tensor("xb", [N, C], BF16, kind="ExternalInput").ap()
    bb = nc.dram_tensor("bb", [N, C], BF16, kind="ExternalInput").ap()
    w1t = nc.dram_tensor("w1t", [C, 2 * C], F32R, kind="ExternalInput").ap()
    w2t = nc.dram_tensor("w2t", [C, 2 * C], F32R, kind="ExternalInput").ap()
    # blocked transposed output layout: [kb, part(c within pair), pair, n-col]
    ox = nc.dram_tensor("oxT", [NBIG, P, CB, 512], BF16, kind="ExternalOutput").ap()
    ob = nc.dram_tensor("obT", [NBIG, P, CB, 512], BF16, kind="ExternalOutput").ap()

    with tile.TileContext(nc) as tc:
        _emit(nc, tc, xb, bb, w1t, w2t, ox, ob)

    nc.compile()
    return nc


def _emit(nc, tc, xb, bb, w1t, w2t, ox, ob):
    from contextlib import ExitStack

    ctx = ExitStack()
    with ctx:
        const = ctx.enter_context(tc.tile_pool(name="const", bufs=1))
        wpool = ctx.enter_context(tc.tile_pool(name="wpool", bufs=1))
        tokp = ctx.enter_context(tc.tile_pool(name="tokp", bufs=10))
        xtp = ctx.enter_context(tc.tile_pool(name="xtp", bufs=1))
        xtbp = ctx.enter_context(tc.tile_pool(name="xtbp", bufs=3))
        gqp = ctx.enter_context(tc.tile_pool(name="gqp", bufs=8))
        smallp = ctx.enter_context(tc.tile_pool(name="smallp", bufs=2))
        fpool = ctx.enter_context(tc.tile_pool(name="fpool", bufs=8))
        bdpool = ctx.enter_context(tc.tile_pool(name="bdpool", bufs=8))
        ostp = ctx.enter_context(tc.tile_pool(name="ostp", bufs=3))
        psG = ctx.enter_context(tc.tile_pool(name="psG", bufs=4, space="PSUM"))
        psT = ctx.enter_context(tc.tile_pool(name="psT", bufs=2, space="PSUM"))
        psO = ctx.enter_context(tc.tile_pool(name="psO", bufs=2, space="PSUM"))

        ident = const.tile([P, P], F32, tag="idf")
        masks.make_identity(nc, ident[:])
        ident_bf = const.tile([P, P], BF16, tag="idb")
        masks.make_identity(nc, ident_bf[:])

        # weights: chunk j (c-rows 128j..128j+128) lives at cols [j*2C, (j+1)*2C)
        w_x = wpool.tile([P, CB * 2 * C], F32R, tag="wx")
        w_b = wpool.tile([P, CB * 2 * C], F32R, tag="wb")

        def load_weights():
            nc.sync.dma_start(
                w_x[:].rearrange("p (j c) -> p j c", j=CB),
                w1t[:, :].rearrange("(j p) c -> p j c", p=P),
            )
            nc.sync.dma_start(
                w_b[:].rearrange("p (j c) -> p j c", j=CB),
                w2t[:, :].rearrange("(j p) c -> p j c", p=P),
            )

        def wchunk(w, j):
            return w[:, j * 2 * C:(j + 1) * 2 * C]

        # transposed x, bf16, persisted: pair block m at cols [m*N, (m+1)*N)
        xT_x = xtp.tile([P, CB * N], BF16, tag="xtx")

        def emit_loads(tok_dram, split_first=False):
            toks = []
            for kb in range(NBIG):
                tokb = tokp.tile([P, 4 * C], BF16, tag="tok", name=f"tok{kb}")
                if kb == 0 and split_first:
                    for sub in range(4):
                        nc.sync.dma_start(
                            tokb[:, sub * C:(sub + 1) * C],
                            tok_dram[kb * 512 + sub * P:kb * 512 + (sub + 1) * P, :],
                        )
                else:
                    nc.sync.dma_start(
                        tokb[:].rearrange("p (s c) -> p s c", s=4),
                        tok_dram[kb * 512:(kb + 1) * 512, :].rearrange(
                            "(s p) c -> p s c", p=P
                        ),
                    )
                toks.append(tokb)
            return toks

        def emit_G_tile(gps, sb, k):
            for m in range(CB):
                o = G_OFF[m]
                nc.tensor.matmul(
                    gps[m][:, o:C], sb[:, m * P:(m + 1) * P], sb[:, o:C],
                    start=(k == 0), stop=(k == NT - 1),
                )

        def emit_T_tile(xT, xt_col, sb):
            tps = psT.tile([P, C], BF16, tag="t", name="tps")
            for m in range(CB):
                nc.tensor.transpose(
                    tps[:, m * P:(m + 1) * P], sb[:, m * P:(m + 1) * P],
                    ident_bf[:],
                )
            nc.vector.tensor_copy(
                xT[:].rearrange("p (m n) -> p m n", m=CB)[
                    :, :, xt_col:xt_col + P],
                tps[:].rearrange("p (m n) -> p m n", m=CB),
            )

        def out_chunk(xT, xt_stride, xt_base, BDs, kb, od):
            """outT for 512 n-cols (tile-group kb): per pair p one matmul
            [c-block p, 512 n]; drain into ost quarter p (split DVE/ACT)."""
            ost = ostp.tile([P, 4 * 512], BF16, tag="ost", name="ost")
            for p in range(CB):
                ops = psO.tile([P, 512], F32, tag="o", name=f"ops{p}")
                nc.tensor.matmul(
                    ops[:], BDs[p][:],
                    xT[:, p * xt_stride + xt_base:p * xt_stride + xt_base + 512],
                    start=True, stop=True,
                )
                if p % 2:
                    nc.scalar.copy(ost[:, p * 512:(p + 1) * 512], ops[:])
                else:
                    nc.vector.tensor_copy(ost[:, p * 512:(p + 1) * 512], ops[:])
            nc.scalar.dma_start(od[kb], ost[:])

        def emit_chain(gps, w):
            """G psum -> g sbuf (with mirrored lower blocks) -> Q -> per-pair
            ctx logits + softmax -> normalized prob tiles fp (sbuf, bf16).
            The BD transposes are deferred (emit_bd) so the PE stream can run
            ahead while softmax finishes on vector/scalar."""
            g_sb = []
            for m in range(CB):
                o = G_OFF[m]
                g = gqp.tile([P, C], F32R, tag="gq", name=f"g{m}")
                if m % 2:
                    nc.scalar.copy(g[:, o:C], gps[m][:, o:C])
                else:
                    nc.vector.tensor_copy(g[:, o:C], gps[m][:, o:C])
                g_sb.append(g)
            # mirror lower blocks (i,j), j < i, from (j,i)^T
            nmir = 0
            for i in range(CB):
                for j in range(G_OFF[i] // P):
                    mps = psT.tile([P, P], F32, tag="t", name="mps")
                    nc.tensor.transpose(
                        mps[:], g_sb[j][:, i * P:(i + 1) * P].bitcast(F32),
                        ident[:],
                    )
                    if nmir % 2:
                        nc.scalar.copy(g_sb[i][:, j * P:(j + 1) * P], mps[:])
                    else:
                        nc.vector.tensor_copy(
                            g_sb[i][:, j * P:(j + 1) * P], mps[:])
                    nmir += 1

            q_sb = [None] * CB
            for i in reversed(range(CB)):
                qp = psO.tile([P, C], F32, tag="o", name=f"qp{i}")
                for j in range(CB):
                    nc.tensor.matmul(
                        qp[:], g_sb[j][:, i * P:(i + 1) * P],
                        wchunk(w, j)[:, 0:C], start=(j == 0), stop=(j == 3),
                    )
                q = gqp.tile([P, C], F32R, tag="gq", name=f"q{i}")
                if i % 2:
                    nc.scalar.copy(q[:], qp[:])
                else:
                    nc.vector.tensor_copy(q[:], qp[:])
                q_sb[i] = q

            fps = []
            for p in range(CB):
                # 256-wide moving window keeps f32r at 1 cyc/row; the diagonal
                # block we need sits at col offset 0 (p<3) or 128 (p=3)
                lo = p * P if p < 3 else 2 * P
                coff = 0 if p < 3 else P
                cps = psO.tile([P, C], F32, tag="o", name=f"cps{p}")
                for j in range(CB):
                    nc.tensor.matmul(
                        cps[:, 0:2 * P],
                        wchunk(w, j)[:, C + p * P:C + (p + 1) * P],
                        q_sb[j][:, lo:lo + 2 * P],
                        start=(j == 0), stop=(j == 3),
                    )
                nm = smallp.tile([P, 1], F32, tag="nm", name="nm")
                sm = smallp.tile([P, 1], F32, tag="sm", name="sm")
                rv = smallp.tile([P, 1], F32, tag="rv", name="rv")
                pp = smallp.tile([P, D], F32, tag="pp", name="pp")
                fp = fpool.tile([P, P], BF16, tag="F", name="fp")
                nc.gpsimd.memset(fp[:], 0.0)
                for dd in range(2):
                    s0 = slice(dd * D, (dd + 1) * D)
                    sc = slice(coff + dd * D, coff + (dd + 1) * D)
                    blk = cps[s0, sc]
                    nc.vector.reduce_max(nm[s0, :], blk, axis=AX.X, negate=True)
                    nc.scalar.activation(
                        pp[s0, :], blk, ACT_EXP, bias=nm[s0, :], scale=1.0,
                        accum_out=sm[s0, :],
                    )
                nc.vector.reciprocal(rv[:], sm[:])
                for dd in range(2):
                    s0 = slice(dd * D, (dd + 1) * D)
                    nc.vector.tensor_scalar_mul(fp[s0, s0], pp[s0, :], rv[s0, :])
                fps.append(fp)
            return fps

        def emit_bd(fps):
            BDs = []
            for p in range(CB):
                bps = psT.tile([P, P], BF16, tag="t", name="bps")
                nc.tensor.transpose(bps[:], fps[p][:], ident_bf[:])
                bd = bdpool.tile([P, P], BF16, tag="bd", name=f"bd{p}")
                nc.vector.tensor_copy(bd[:], bps[:])
                BDs.append(bd)
            return BDs

        # ---- schedule ----
        toks_x = emit_loads(xb, split_first=True)
        load_weights()
        toks_b = emit_loads(bb)

        # phase A: interleaved G_x + T_x per tile
        gps_x = [psG.tile([P, C], F32, tag="g", name=f"gpsx{m}") for m in range(CB)]
        for kb in range(NBIG):
            for sub in range(4):
                k = kb * 4 + sub
                sb = toks_x[kb][:, sub * C:(sub + 1) * C]
                emit_G_tile(gps_x, sb, k)
                emit_T_tile(xT_x, k * P, sb)
        f1 = emit_chain(gps_x, w_x)

        # phase B: G_b + T_b per kb, with out_b pipelined one kb behind so the
        # PE stream never waits on softmax (bd1) or the DVE xT drains
        gps_b = [psG.tile([P, C], F32, tag="g", name=f"gpsb{m}") for m in range(CB)]
        bd1 = None
        xtb_tiles = []
        for kb in range(NBIG):
            xtb = xtbp.tile([P, CB * 512], BF16, tag="xtb", name="xtb")
            xtb_tiles.append(xtb)
            for sub in range(4):
                k = kb * 4 + sub
                sb = toks_b[kb][:, sub * C:(sub + 1) * C]
                emit_G_tile(gps_b, sb, k)
                emit_T_tile(xtb, sub * P, sb)
            if kb == 0:
                bd1 = emit_bd(f1)  # softmax done by now; PE was busy on kb=0
            else:
                out_chunk(xtb_tiles[kb - 1], 512, 0, bd1, kb - 1, ob)
        out_chunk(xtb_tiles[NBIG - 1], 512, 0, bd1, NBIG - 1, ob)
        f2 = emit_chain(gps_b, w_b)
        bd2 = emit_bd(f2)

        # phase C: out_x from the persisted xT_x
        for kb in range(NBIG):
            out_chunk(xT_x, N, kb * 512, bd2, kb, ox)


_NC_CACHE = None


def _get_nc():
    global _NC_CACHE
    if _NC_CACHE is None:
        _NC_CACHE = build_nc()
    return _NC_CACHE


def _prep_inputs(x, blood, W1, W2):
    x16 = np.asarray(x, dtype=np.float32).astype(ml_dtypes.bfloat16)
    b16 = np.asarray(blood, dtype=np.float32).astype(ml_dtypes.bfloat16)
    w1t = np.ascontiguousarray(np.asarray(W1, dtype=np.float32).T)
    w2t = np.ascontiguousarray(np.asarray(W2, dtype=np.float32).T)
    w1t[:, :C] *= SCALE  # fold softmax scale into the k-projection (exact: 2^-3)
    w2t[:, :C] *= SCALE
    return [
        {"xb": np.ascontiguousarray(x16[b]), "bb": np.ascontiguousarray(b16[b]),
         "w1t": w1t, "w2t": w2t} for b in range(B)
    ]


def _unshuffle(arr):
    """[NBIG, P, CB, 512] blocked-transposed bf16 -> [N, C] natural f32."""
    # arr[kb, part, p, col] = out[kb*512 + col, p*128 + part]
    return np.ascontiguousarray(
        arr.transpose(0, 3, 2, 1).reshape(N, C).astype(np.float32))


def kernel(x, blood, W1, W2, trace=False):
    nc = _get_nc()
    in_maps = _prep_inputs(x, blood, W1, W2)
    res = run_bass_kernel_spmd(nc, in_maps, core_ids=list(range(B)), trace=trace)
    out_x = np.stack([_unshuffle(res.results[b]["oxT"]) for b in range(B)])
    out_b = np.stack([_unshuffle(res.results[b]["obT"]) for b in range(B)])
    if trace:
        kernel.last_results = res
    return (out_x, out_b)
